# revision 1
# baseline (speedup 1.0000x reference)
"""RNN-T Joint network kernel for Trainium2 (Bass/Tile), 8-core data-parallel.

Math (per batch b):
  hf = f[b] @ W1[:1024]            # (T=256, J=640)
  hg = g[b] @ W1[1024:]            # (U=65,  J=640)
  h[t,u,:]   = relu(hf[t] + hg[u] + b1)
  out[t,u,:] = h[t,u,:] @ W2 + b2  # (256, 65, 1024)

Device layout (per core, u-major):
  - hfT[j, t] and hgT'[j, u] = hgT + b1 kept resident in SBUF (j on partitions).
  - For each u: H_u[j, t] = relu(hfT[j, t] + hgT'[j, u]) built by ScalarE
    (bias = per-partition column hgT'[:, u]), cast to bf16.
  - PE: out_tile[t128, v512] += H_u[jc][:, t128].T @ W2bf[jc][:, v512], 5 j-chunks
    accumulated in PSUM (fp32).
  - VectorE drains PSUM + adds broadcast b2, DMA straight to HBM.
"""

import numpy as np

T, U = 256, 65
EH, PH, J, V = 1024, 320, 640, 1024
JC = J // 128           # 5 j-chunks
HC = EH // 128          # 8 h-chunks
N_CORES = 8

_CACHE = {}


def _build_nc():
    import concourse.bass as bass
    import concourse.bacc as bacc
    import concourse.mybir as mybir
    from concourse import tile, masks

    f32 = mybir.dt.float32
    bf16 = mybir.dt.bfloat16
    Relu = mybir.ActivationFunctionType.Relu
    add = mybir.AluOpType.add

    nc = bacc.Bacc(None, target_bir_lowering=False)

    f_d = nc.declare_dram_parameter("f", [T, EH], f32, isOutput=False)
    g_d = nc.declare_dram_parameter("g", [U, PH], f32, isOutput=False)
    W1_d = nc.declare_dram_parameter("W1", [EH + PH, J], f32, isOutput=False)
    b1_d = nc.declare_dram_parameter("b1", [J], f32, isOutput=False)
    W2_d = nc.declare_dram_parameter("W2", [J, V], f32, isOutput=False)
    b2_d = nc.declare_dram_parameter("b2", [V], f32, isOutput=False)
    out_d = nc.declare_dram_parameter("out", [T, U, V], f32, isOutput=True)

    # W1g partition chunks (PH = 320 = 128 + 128 + 64)
    g_chunks = [(0, 128), (128, 128), (256, 64)]

    with tile.TileContext(nc) as tc:
        with tc.tile_pool(name="const", bufs=1) as cpool:
            identity = cpool.tile([128, 128], f32)
            masks.make_identity(nc, identity[:])

            # ---------------- prologue: weights + first layer ----------------
            W2b = []      # bf16 [128, V] x JC
            fTb = []      # bf16 [128, T] x HC   (f^T)
            hfTs = []     # f32  [128, T] x JC   (hf^T)
            hgTs = []     # f32  [128, U] x JC   (hg^T + b1)

            with (
                tc.tile_pool(name="scratch", bufs=2) as spool,
                tc.tile_pool(name="ppsumA", bufs=2, space=bass.MemorySpace.PSUM) as ppA,
                tc.tile_pool(name="ppsumB", bufs=1, space=bass.MemorySpace.PSUM) as ppB,
                tc.tile_pool(name="ppsumC", bufs=2, space=bass.MemorySpace.PSUM) as ppC,
            ):
                # f first: longest dependency chain (load -> transpose -> hfT)
                fraw = []
                for tt in range(2):
                    t = spool.tile([128, EH], f32, tag=f"fraw{tt}")
                    nc.sync.dma_start(out=t[:], in_=f_d[tt * 128:(tt + 1) * 128, :])
                    fraw.append(t)

                # W1f -> bf16  (rows 0:1024 of W1)
                W1fb = []
                for h in range(HC):
                    w1raw = spool.tile([128, J], f32, tag="w1raw")
                    nc.sync.dma_start(out=w1raw[:], in_=W1_d[h * 128:(h + 1) * 128, :])
                    t = cpool.tile([128, J], bf16, tag=f"w1fb{h}")
                    nc.vector.tensor_copy(t[:], w1raw[:])
                    W1fb.append(t)

                # f -> f^T (PE transpose, fp32 in -> psum -> bf16 sbuf)
                for h in range(HC):
                    ft = cpool.tile([128, T], bf16, tag=f"fT{h}")
                    for tt in range(2):
                        pt = ppA.tile([128, 128], f32, tag="tp")
                        nc.tensor.transpose(pt[:], fraw[tt][:, h * 128:(h + 1) * 128],
                                            identity[:])
                        nc.vector.tensor_copy(ft[:, tt * 128:(tt + 1) * 128], pt[:])
                    fTb.append(ft)

                # hf^T = W1f^T @ f^T
                for c in range(JC):
                    pf = ppC.tile([128, T], f32, tag="pf")
                    for h in range(HC):
                        nc.tensor.matmul(pf[:], W1fb[h][:, c * 128:(c + 1) * 128],
                                         fTb[h][:], start=(h == 0), stop=(h == HC - 1))
                    t = cpool.tile([128, T], f32, tag=f"hfT{c}")
                    nc.vector.tensor_copy(t[:], pf[:])
                    hfTs.append(t)

                # W2 -> bf16
                for c in range(JC):
                    w2raw = spool.tile([128, V], f32, tag="w2raw")
                    nc.sync.dma_start(out=w2raw[:], in_=W2_d[c * 128:(c + 1) * 128, :])
                    t = cpool.tile([128, V], bf16, tag=f"w2b{c}")
                    nc.vector.tensor_copy(t[:], w2raw[:])
                    W2b.append(t)

                # W1g -> bf16  (rows 1024:1344)
                W1gb = []
                for pc, (po, pn) in enumerate(g_chunks):
                    w1graw = spool.tile([pn, J], f32, tag="w1graw")
                    nc.sync.dma_start(out=w1graw[:], in_=W1_d[EH + po:EH + po + pn, :])
                    t = cpool.tile([pn, J], bf16, tag=f"w1gb{pc}")
                    nc.vector.tensor_copy(t[:], w1graw[:])
                    W1gb.append(t)

                # b1 as [128, JC] (partition p, chunk c) ; b2 row
                b1sb = cpool.tile([128, JC], f32)
                nc.sync.dma_start(out=b1sb[:], in_=b1_d[:].rearrange("(c p) -> p c", p=128))
                b2row = cpool.tile([1, V], f32)
                nc.sync.dma_start(out=b2row[:], in_=b2_d[:].rearrange("(a v) -> a v", a=1))

                # broadcast b2 across 128 partitions via rank-1 matmul
                ones = cpool.tile([1, 128], f32)
                nc.vector.memset(ones[:], 1.0)
                b2bc = cpool.tile([128, V], f32)
                for vh in range(2):
                    pb = ppB.tile([128, 512], f32, tag="pb")
                    nc.tensor.matmul(pb[:], ones[:], b2row[:, vh * 512:(vh + 1) * 512],
                                     start=True, stop=True)
                    nc.vector.tensor_copy(b2bc[:, vh * 512:(vh + 1) * 512], pb[:])

                # g -> g^T
                graw = spool.tile([U, PH], f32, tag="graw")
                nc.sync.dma_start(out=graw[:], in_=g_d[:])
                gTb = []
                for pc, (po, pn) in enumerate(g_chunks):
                    pt = ppA.tile([128, U], f32, tag="tp2")
                    nc.tensor.transpose(pt[:pn, :], graw[:, po:po + pn],
                                        identity[:U, :U])
                    t = cpool.tile([pn, U], bf16, tag=f"gT{pc}")
                    nc.vector.tensor_copy(t[:], pt[:pn, :])
                    gTb.append(t)

                # hg^T = W1g^T @ g^T  (+ b1, fused on drain)
                for c in range(JC):
                    ph = ppB.tile([128, U], f32, tag="ph")
                    for pc in range(3):
                        nc.tensor.matmul(ph[:], W1gb[pc][:, c * 128:(c + 1) * 128],
                                         gTb[pc][:], start=(pc == 0), stop=(pc == 2))
                    t = cpool.tile([128, U], f32, tag=f"hgT{c}")
                    nc.vector.tensor_scalar(t[:], ph[:], b1sb[:, c:c + 1], None, add)
                    hgTs.append(t)

            # ---------------- main loop over u ----------------
            with (
                tc.tile_pool(name="hpool", bufs=4) as hpool,
                tc.tile_pool(name="opool", bufs=4) as opool,
                tc.tile_pool(name="mpsum", bufs=2, space=bass.MemorySpace.PSUM) as mpsum,
            ):
                for u in range(U):
                    Hs = []
                    for c in range(JC):
                        ht = hpool.tile([128, T], bf16, tag=f"H{c}")
                        nc.scalar.activation(ht[:], hfTs[c][:], Relu,
                                             bias=hgTs[c][:, u:u + 1], scale=1.0)
                        Hs.append(ht)
                    for tt in range(2):
                        ps0 = mpsum.tile([128, 512], f32, tag=f"ps{tt}0")
                        ps1 = mpsum.tile([128, 512], f32, tag=f"ps{tt}1")
                        ps = [ps0, ps1]
                        for c in range(JC):
                            lhsT = Hs[c][:, tt * 128:(tt + 1) * 128]
                            nc.tensor.matmul(ps[0][:], lhsT, W2b[c][:, 0:512],
                                             start=(c == 0), stop=(c == JC - 1))
                            nc.tensor.matmul(ps[1][:], lhsT, W2b[c][:, 512:1024],
                                             start=(c == 0), stop=(c == JC - 1))
                        for vh in range(2):
                            ot = opool.tile([128, 512], f32, tag=f"o{tt}{vh}")
                            nc.vector.tensor_tensor(
                                ot[:], ps[vh][:],
                                b2bc[:, vh * 512:(vh + 1) * 512], add)
                            nc.sync.dma_start(
                                out=out_d[tt * 128:(tt + 1) * 128, u,
                                          vh * 512:(vh + 1) * 512],
                                in_=ot[:])
    nc.compile()
    return nc


def _get_nc():
    if "nc" not in _CACHE:
        _CACHE["nc"] = _build_nc()
    return _CACHE["nc"]


def run(f, g, W1, b1, W2, b2, trace=False):
    """Returns (full_output, BassKernelResults)."""
    from concourse.bass_utils import run_bass_kernel_spmd

    nc = _get_nc()
    in_maps = []
    for i in range(N_CORES):
        in_maps.append({
            "f": np.ascontiguousarray(f[i], dtype=np.float32),
            "g": np.ascontiguousarray(g[i], dtype=np.float32),
            "W1": np.ascontiguousarray(W1, dtype=np.float32),
            "b1": np.ascontiguousarray(b1, dtype=np.float32),
            "W2": np.ascontiguousarray(W2, dtype=np.float32),
            "b2": np.ascontiguousarray(b2, dtype=np.float32),
        })
    res = run_bass_kernel_spmd(nc, in_maps, list(range(N_CORES)), trace=trace)
    out = np.stack([res.results[i]["out"] for i in range(N_CORES)], axis=0)
    return out, res


def kernel(f, g, W1, b1, W2, b2):
    out, _ = run(f, g, W1, b1, W2, b2)
    return out



# revision 24
# speedup vs baseline: 1.1860x; 1.1860x over previous
"""RNN-T Joint network kernel for Trainium2 (Bass/Tile), 8-core data-parallel.

Math (per batch b):
  hf = f[b] @ W1[:1024]            # (T=256, J=640)
  hg = g[b] @ W1[1024:] + b1       # (U=65,  J=640)
  a[t,u,:]   = hf[t] + hg[u]
  out[t,u,:] = relu(a) @ W2 + b2   # (256, 65, 1024)

Split relu(a) = 0.5*a + (0.5|a| - beta) + beta  (per-j constant beta):
  out = F2[t] + G2[u] + r @ W2,  r = 0.5|a| - beta  (zero-mean-ish residual)
where F2 = 0.5*hf@W2 (t,v), G2 = (0.5*(hg+b1)+beta)@W2 + b2 (u,v).

Device strategy (per core, u-major, psum scale 2^15):
  - r computed as |2^4 a| - 2^5 beta -> fp8 e4m3 on DVE (1-2 tensor_scalar
    ops per j-chunk; chunk 4 left uncentered so it casts in one op).
  - Main matmul in fp8 DoubleRow (2 j-chunks per PE instruction, 0.5 cyc/row):
    W2 quantized e4m3 * 2^10.
  - G2 rides the chunk-4 DR matmul as 4 extra contraction rows (3-term fp8
    ladder qA,qA,qB,qC with coeffs 2^7,2^7,2^4,1 -> ~1e-4 abs error).
  - F2 rides a DoubleRow *identity* matmul: lhsT = (128*I, 8*I) fp8 pair,
    rhs = (F2hi, F2lo) 2-term fp8 split of 2^15*F2.
  - Drains are then pure psum->bf16 copies: Act takes vh0, Pool takes vh1.
  - Output bf16 [T,U,V] (halves DMA bytes); host upcasts fp32 * 2^-15 exact.
"""

import numpy as np

T, U = 256, 65
EH, PH, J, V = 1024, 320, 640, 1024
HC = EH // 128          # 8 h-chunks
N_CORES = 8
PSC = 2.0 ** -15        # host-side inverse psum scale

_CACHE = {}


def _build_nc():
    import concourse.bass as bass
    import concourse.bacc as bacc
    import concourse.mybir as mybir
    from concourse import tile, masks

    f32 = mybir.dt.float32
    bf16 = mybir.dt.bfloat16
    fp8 = mybir.dt.float8e4
    Square = mybir.ActivationFunctionType.Square
    Sqrt = mybir.ActivationFunctionType.Sqrt
    Copy = mybir.ActivationFunctionType.Copy
    Identity = mybir.ActivationFunctionType.Identity
    add = mybir.AluOpType.add
    sub = mybir.AluOpType.subtract
    mult = mybir.AluOpType.mult
    mx = mybir.AluOpType.max
    DR = mybir.MatmulPerfMode.DoubleRow

    nc = bacc.Bacc(None, target_bir_lowering=False)

    f_d = nc.declare_dram_parameter("f", [T, EH], f32, isOutput=False)
    g_d = nc.declare_dram_parameter("g", [U, PH], f32, isOutput=False)
    W1_d = nc.declare_dram_parameter("W1", [EH + PH, J], f32, isOutput=False)
    b1_d = nc.declare_dram_parameter("b1", [J], f32, isOutput=False)
    W2_d = nc.declare_dram_parameter("W2", [J, V], f32, isOutput=False)
    b2_d = nc.declare_dram_parameter("b2", [V], f32, isOutput=False)
    out_d = nc.declare_dram_parameter("out", [T, U, V], bf16, isOutput=True)
    qstage_d = nc.dram_tensor("qstage", [U, 4, V], fp8)
    cstage_d = nc.dram_tensor("cstage", [1, 4 * T], fp8)

    # W1g partition chunks (PH = 320 = 128 + 128 + 64)
    g_chunks = [(0, 128), (128, 128), (256, 64)]

    with tile.TileContext(nc) as tc:
        with tc.tile_pool(name="const", bufs=1) as cpool:
            identity = cpool.tile([128, 128], f32)
            masks.make_identity(nc, identity[:])

            # ------------------------------------------------------------
            # Prologue: loads, first layer, stats, F2/G2, fp8 weight prep
            # ------------------------------------------------------------
            hfT4 = []      # bf16 [128,256] x4 : 2^4 hf^T   (chunks 0-3)
            hgT4 = []      # f32  [128,65]  x4 : 2^4 (hg+b1)^T
            betac = []     # f32  [128,1]   x4 : 2^5 beta
            W2s = []       # bf16 [128,1024] x4 + 2x[64,1024] : 2^7 W2

            with (
                tc.tile_pool(name="scratch", bufs=2) as spool,
                tc.tile_pool(name="prot", bufs=1) as ppro,
            ):
                # ---- loads ----
                fraw = []
                for tt in range(2):
                    t = spool.tile([128, EH], f32, tag=f"fraw{tt}")
                    nc.sync.dma_start(out=t[:], in_=f_d[tt * 128:(tt + 1) * 128, :])
                    fraw.append(t)

                w1raw = []
                for h in range(HC):
                    t = spool.tile([128, J], f32, tag="w1raw")
                    nc.sync.dma_start(out=t[:], in_=W1_d[h * 128:(h + 1) * 128, :])
                    w1raw.append(t)

                w2raw = []
                for c in range(4):
                    t = spool.tile([128, V], f32, tag="w2raw")
                    nc.sync.dma_start(out=t[:], in_=W2_d[c * 128:(c + 1) * 128, :])
                    w2raw.append(t)
                w2raw4 = []
                for i in range(2):
                    t = spool.tile([64, V], f32, tag=f"w2raw4{i}")
                    nc.sync.dma_start(
                        out=t[:], in_=W2_d[512 + 64 * i:512 + 64 * (i + 1), :])
                    w2raw4.append(t)

                graw = ppro.tile([U, PH], f32)
                nc.sync.dma_start(out=graw[:], in_=g_d[:])

                b1A = ppro.tile([128, 5], f32)
                nc.sync.dma_start(out=b1A[:], in_=b1_d[:].rearrange("(c p) -> p c", p=128))
                b1B = ppro.tile([64, 1], f32)
                nc.sync.dma_start(out=b1B[:], in_=b1_d[576:640].rearrange("(a p) -> p a", a=1))
                b2row = ppro.tile([1, V], f32)
                nc.sync.dma_start(out=b2row[:], in_=b2_d[:].rearrange("(a v) -> a v", a=1))

                # scaled b1 (2^4)
                b1A16 = ppro.tile([128, 5], f32)
                nc.vector.tensor_scalar(b1A16[:], b1A[:], 16.0, None, mult)
                b1B16 = ppro.tile([64, 1], f32)
                nc.vector.tensor_scalar(b1B16[:], b1B[:], 16.0, None, mult)

                # ---- W1 -> bf16 ----
                W1fb = []
                for h in range(HC):
                    t = cpool.tile([128, J], bf16, tag=f"w1fb{h}")
                    nc.vector.tensor_copy(t[:], w1raw[h][:])
                    W1fb.append(t)
                W1gb = []
                for pc, (po, pn) in enumerate(g_chunks):
                    w1graw = spool.tile([pn, J], f32, tag="w1graw")
                    nc.sync.dma_start(out=w1graw[:], in_=W1_d[EH + po:EH + po + pn, :])
                    t = ppro.tile([pn, J], bf16, tag=f"w1gb{pc}")
                    nc.vector.tensor_copy(t[:], w1graw[:])
                    W1gb.append(t)

                # ---- f^T, g^T (PE transpose) + b2 broadcast ----
                fTb = []
                gTb = []
                b2bc = ppro.tile([U, V], f32)
                with tc.tile_pool(name="ppA", bufs=2,
                                  space=bass.MemorySpace.PSUM) as ppA:
                    for h in range(HC):
                        ft = ppro.tile([128, T], bf16, tag=f"fT{h}")
                        for tt in range(2):
                            pt = ppA.tile([128, 128], f32, tag="tp")
                            nc.tensor.transpose(pt[:],
                                                fraw[tt][:, h * 128:(h + 1) * 128],
                                                identity[:])
                            nc.vector.tensor_copy(ft[:, tt * 128:(tt + 1) * 128],
                                                  pt[:])
                        fTb.append(ft)
                    for pc, (po, pn) in enumerate(g_chunks):
                        pt = ppA.tile([128, U], f32, tag="tp2")
                        nc.tensor.transpose(pt[:pn, :], graw[:, po:po + pn],
                                            identity[:U, :U])
                        t = ppro.tile([pn, U], bf16, tag=f"gT{pc}")
                        nc.vector.tensor_copy(t[:], pt[:pn, :])
                        gTb.append(t)
                    onesU = ppro.tile([1, U], f32)
                    nc.vector.memset(onesU[:], 1.0)
                    for vh in range(2):
                        pb = ppA.tile([U, 512], f32, tag="pb")
                        nc.tensor.matmul(pb[:], onesU[:],
                                         b2row[:, vh * 512:(vh + 1) * 512],
                                         start=True, stop=True)
                        nc.vector.tensor_copy(b2bc[:, vh * 512:(vh + 1) * 512],
                                              pb[:])

                # ---- hf^T (2^4 scale), chunks 0-3 full + chunk4 as two halves
                hfT4h = []   # [64,256] x2 (chunk4 j 512-575, 576-639 at part 0-63)
                hgT4h = []
                with tc.tile_pool(name="ppB", bufs=2,
                                  space=bass.MemorySpace.PSUM) as ppB:
                    for c in range(4):
                        pf = ppB.tile([128, T], f32, tag="pf")
                        for h in range(HC):
                            nc.tensor.matmul(pf[:], W1fb[h][:, c * 128:(c + 1) * 128],
                                             fTb[h][:], start=(h == 0),
                                             stop=(h == HC - 1))
                        t = cpool.tile([128, T], bf16, tag=f"hfT4_{c}")
                        nc.vector.tensor_scalar(t[:], pf[:], 16.0, None, mult)
                        hfT4.append(t)
                    for i in range(2):
                        pf = ppB.tile([64, T], f32, tag="pf4")
                        lo = 512 + 64 * i
                        for h in range(HC):
                            nc.tensor.matmul(pf[:], W1fb[h][:, lo:lo + 64],
                                             fTb[h][:], start=(h == 0),
                                             stop=(h == HC - 1))
                        t = cpool.tile([64, T], bf16, tag=f"hfT4h{i}")
                        nc.vector.tensor_scalar(t[:], pf[:], 16.0, None, mult)
                        hfT4h.append(t)

                    # ---- hg^T + b1 (2^4 scale)
                    for c in range(4):
                        ph = ppB.tile([128, U], f32, tag="ph")
                        for pc in range(3):
                            nc.tensor.matmul(ph[:], W1gb[pc][:, c * 128:(c + 1) * 128],
                                             gTb[pc][:], start=(pc == 0),
                                             stop=(pc == 2))
                        t = cpool.tile([128, U], f32, tag=f"hgT4_{c}")
                        nc.vector.tensor_scalar(t[:], ph[:], 16.0, b1A16[:, c:c + 1],
                                                mult, add)
                        hgT4.append(t)
                    for i in range(2):
                        ph = ppB.tile([64, U], f32, tag="ph4")
                        lo = 512 + 64 * i
                        for pc in range(3):
                            nc.tensor.matmul(ph[:], W1gb[pc][:, lo:lo + 64],
                                             gTb[pc][:], start=(pc == 0),
                                             stop=(pc == 2))
                        t = cpool.tile([64, U], f32, tag=f"hgT4h{i}")
                        bcol = b1A16[0:64, 4:5] if i == 0 else b1B16[:, 0:1]
                        nc.vector.tensor_scalar(t[:], ph[:], 16.0, bcol, mult, add)
                        hgT4h.append(t)

                # ---- per-j stats -> beta (chunks 0-3 only; chunk4 uncentered)
                sqf = ppro.tile([128, T], f32)
                sqg = ppro.tile([128, U], f32)
                for c in range(4):
                    m2f = ppro.tile([128, 1], f32, tag=f"m2f{c}")
                    nc.scalar.activation(sqf[:], hfT4[c][:], Square, accum_out=m2f[:])
                    m2g = ppro.tile([128, 1], f32, tag=f"m2g{c}")
                    nc.scalar.activation(sqg[:], hgT4[c][:], Square, accum_out=m2g[:])
                    m1f = ppro.tile([128, 1], f32, tag=f"m1f{c}")
                    nc.vector.tensor_reduce(m1f[:], hfT4[c][:],
                                            mybir.AxisListType.X, add)
                    m1g = ppro.tile([128, 1], f32, tag=f"m1g{c}")
                    nc.vector.tensor_reduce(m1g[:], hgT4[c][:],
                                            mybir.AxisListType.X, add)
                    # msq8 = m2f/T + m2g/U + (2/(T*U)) m1f m1g   (= 2^8 E[a^2])
                    msq = ppro.tile([128, 1], f32, tag=f"msq{c}")
                    nc.vector.tensor_tensor(msq[:], m1f[:], m1g[:], mult)
                    nc.vector.tensor_scalar(msq[:], msq[:], 2.0 / (T * U), None, mult)
                    t1 = ppro.tile([128, 1], f32, tag=f"t1_{c}")
                    nc.vector.tensor_scalar(t1[:], m2f[:], 1.0 / T, None, mult)
                    nc.vector.tensor_tensor(msq[:], msq[:], t1[:], add)
                    nc.vector.tensor_scalar(t1[:], m2g[:], 1.0 / U, None, mult)
                    nc.vector.tensor_tensor(msq[:], msq[:], t1[:], add)
                    # betac = 2^5 beta = sqrt((2/pi) * msq8)
                    bc = cpool.tile([128, 1], f32, tag=f"beta{c}")
                    nc.scalar.activation(bc[:], msq[:], Sqrt, bias=0.0,
                                         scale=float(2.0 / np.pi))
                    betac.append(bc)

                # ---- W2 -> bf16 (2^7) and fp8 (2^10)
                for c in range(4):
                    t = cpool.tile([128, V], bf16, tag=f"w2s{c}")
                    nc.vector.tensor_scalar(t[:], w2raw[c][:], 128.0, None, mult)
                    W2s.append(t)
                W2s4 = []
                for i in range(2):
                    t = cpool.tile([64, V], bf16, tag=f"w2s4{i}")
                    nc.vector.tensor_scalar(t[:], w2raw4[i][:], 128.0, None, mult)
                    W2s4.append(t)

                W2Qp = []    # fp8 [128,2,1024] x2 : pairs (0,1), (2,3)
                for p in range(2):
                    t = cpool.tile([128, 2, V], fp8, tag=f"w2qp{p}")
                    for i in range(2):
                        nc.vector.tensor_scalar(t[:, i, :], w2raw[2 * p + i][:],
                                                1024.0, None, mult)
                    W2Qp.append(t)
                W2Q3 = []    # fp8 [66,2,1024] x4 (4-deep buffer over u so the
                # per-u Q-row DMA runs well ahead of its consumer)
                for ver in range(4):
                    t = cpool.tile([66, 2, V], fp8, tag=f"w2q3_{ver}")
                    for i in range(2):
                        nc.vector.tensor_scalar(t[0:64, i, :], w2raw4[i][:],
                                                1024.0, None, mult)
                    W2Q3.append(t)

                # ---- identity pair for the F2 DoubleRow add: (128 I, 8 I)
                Ipair = cpool.tile([128, 2, 128], fp8)
                nc.vector.tensor_scalar(Ipair[:, 0, :], identity[:], 128.0, None, mult)
                nc.vector.tensor_scalar(Ipair[:, 1, :], identity[:], 8.0, None, mult)

                # ---- F2 / G2 psums ----
                F2pair = []
                g2s = ppro.tile([U, V], f32)      # 2^7 G2'
                with tc.tile_pool(name="ppC", bufs=2,
                                  space=bass.MemorySpace.PSUM) as ppC:
                    # F2 = psum(2^11 hf@W2) -> 2-term fp8 pair (2^15*0.5*hf@W2)
                    for tt in range(2):
                        f2f = ppro.tile([128, V], f32, tag=f"f2f{tt}")
                        for vh in range(2):
                            pF = ppC.tile([128, 512], f32, tag="pF")
                            vs = slice(vh * 512, (vh + 1) * 512)
                            ts_ = slice(tt * 128, (tt + 1) * 128)
                            for c in range(4):
                                nc.tensor.matmul(pF[:], hfT4[c][:, ts_], W2s[c][:, vs],
                                                 start=(c == 0), stop=False)
                            for i in range(2):
                                nc.tensor.matmul(pF[:], hfT4h[i][:, ts_],
                                                 W2s4[i][:, vs],
                                                 start=False, stop=(i == 1))
                            nc.scalar.activation(f2f[:, vs], pF[:], Copy, bias=0.0,
                                                 scale=1.0)
                        fp = cpool.tile([128, 2, V], fp8, tag=f"f2pair{tt}")
                        # hi = e4m3(2^-4 * psum); lo = e4m3(psum - 2^4 hi)
                        nc.vector.tensor_scalar(fp[:, 0, :], f2f[:], 0.0625, None,
                                                mult)
                        hid = ppro.tile([128, V], f32, tag=f"f2hid{tt}")
                        nc.vector.tensor_scalar(hid[:], fp[:, 0, :], 16.0, None, mult)
                        dd = ppro.tile([128, V], f32, tag=f"f2dd{tt}")
                        nc.vector.tensor_tensor(dd[:], f2f[:], hid[:], sub)
                        nc.vector.tensor_copy(fp[:, 1, :], dd[:])
                        F2pair.append(fp)

                    # G2' = (0.5(hg+b1)+beta)@W2 + b2 (stored as 2^7 G2')
                    Xg = []
                    for c in range(4):
                        bh = ppro.tile([128, 1], f32, tag=f"bh{c}")
                        nc.vector.tensor_scalar(bh[:], betac[c][:], 0.5, None, mult)
                        t = ppro.tile([128, U], bf16, tag=f"xg{c}")
                        nc.vector.tensor_scalar(t[:], hgT4[c][:], 0.5, bh[:], mult,
                                                add)
                        Xg.append(t)
                    Xg4 = []
                    for i in range(2):
                        t = ppro.tile([64, U], bf16, tag=f"xg4{i}")
                        nc.vector.tensor_scalar(t[:], hgT4h[i][:], 0.5, None, mult)
                        Xg4.append(t)

                    for vh in range(2):
                        pG = ppC.tile([U, 512], f32, tag="pG")
                        vs = slice(vh * 512, (vh + 1) * 512)
                        for c in range(4):
                            nc.tensor.matmul(pG[:], Xg[c][:], W2s[c][:, vs],
                                             start=(c == 0), stop=False)
                        for i in range(2):
                            nc.tensor.matmul(pG[:], Xg4[i][:], W2s4[i][:, vs],
                                             start=False, stop=(i == 1))
                        # g2s = 2^-4 psum(2^11 G2'-b2part) + 2^7 b2
                        nc.vector.tensor_scalar(g2s[:, vs], pG[:], 0.0625, None, mult)
                nc.vector.tensor_scalar(b2bc[:], b2bc[:], 128.0, None, mult)
                nc.vector.tensor_tensor(g2s[:], g2s[:], b2bc[:], add)

                # build-pass helpers: P = a-b = hfT4 + (hg4-b);
                # Q = -a-b = (-hfT4) + (-hg4-b); r8 = max(P, Q) = |a|-b
                cPm = []
                cQn = []
                nhfT4 = []
                nhfT4h = []
                for c in range(4):
                    t = cpool.tile([128, U], f32, tag=f"cPm{c}")
                    nc.vector.tensor_scalar(t[:], hgT4[c][:], betac[c][:],
                                            None, sub)
                    cPm.append(t)
                    t2_ = cpool.tile([128, U], f32, tag=f"cQn{c}")
                    nc.vector.tensor_scalar(t2_[:], hgT4[c][:], -1.0,
                                            betac[c][:], mult, sub)
                    cQn.append(t2_)
                    t3_ = cpool.tile([128, T], bf16, tag=f"nhf{c}")
                    nc.vector.tensor_scalar(t3_[:], hfT4[c][:], -1.0, None,
                                            mult)
                    nhfT4.append(t3_)
                cQn4 = []
                for i in range(2):
                    t = cpool.tile([64, U], f32, tag=f"cQn4{i}")
                    nc.vector.tensor_scalar(t[:], hgT4h[i][:], -1.0, None,
                                            mult)
                    cQn4.append(t)
                    t2_ = cpool.tile([64, T], bf16, tag=f"nhf4{i}")
                    nc.vector.tensor_scalar(t2_[:], hfT4h[i][:], -1.0, None,
                                            mult)
                    nhfT4h.append(t2_)

                qall = ppro.tile([U, 4, V], fp8)     # (qA, qA, qB, qC) per u
                nc.vector.tensor_copy(qall[:, 0, :], g2s[:])
                nc.vector.tensor_copy(qall[:, 1, :], qall[:, 0, :])
                qad = ppro.tile([U, V], f32)
                nc.vector.tensor_scalar(qad[:], qall[:, 0, :], 1.0, None, mult)
                d1 = ppro.tile([U, V], f32)
                nc.vector.tensor_tensor(d1[:], g2s[:], qad[:], sub)
                nc.vector.tensor_scalar(qall[:, 2, :], d1[:], 16.0, None, mult)
                qbd = ppro.tile([U, V], f32)
                nc.vector.tensor_scalar(qbd[:], qall[:, 2, :], 0.0625, None, mult)
                d2 = ppro.tile([U, V], f32)
                nc.vector.tensor_tensor(d2[:], d1[:], qbd[:], sub)
                nc.vector.tensor_scalar(qall[:, 3, :], d2[:], 256.0, None, mult)
                # stage to DRAM scratch so the per-u row injection can reshape
                # one u-row into 2 SBUF partitions (SBUF->SBUF can't)
                nc.sync.dma_start(out=qstage_d[:, :, :], in_=qall[:, :, :])

                # ---- r8 chunk-4 lhsT tiles with constant coeff rows
                # (engine ops must base at partition 0/32/64/96, so stage the
                #  two coeff rows via DRAM and DMA them into rows 64-65)
                coefstage = ppro.tile([1, 4, T], fp8)
                nc.vector.memset(coefstage[0:1, 0, :], 128.0)   # row64 i0: qA
                nc.vector.memset(coefstage[0:1, 1, :], 128.0)   # row64 i1: qA
                nc.vector.memset(coefstage[0:1, 2, :], 16.0)    # row65 i0: qB
                nc.vector.memset(coefstage[0:1, 3, :], 1.0)     # row65 i1: qC
                nc.sync.dma_start(out=cstage_d[:, :],
                                  in_=coefstage[0:1, :, :].rearrange(
                                      "a i v -> a (i v)"))
                r8c4 = []
                for ver in range(3):
                    t = cpool.tile([66, 2, T], fp8, tag=f"r8c4_{ver}")
                    nc.sync.dma_start(
                        out=t[64:66, :, :],
                        in_=cstage_d[:, :].rearrange(
                            "a (p i v) -> (a p) i v", p=2, i=2))
                    r8c4.append(t)

            # ------------------------------------------------------------
            # Main loop over u
            # ------------------------------------------------------------
            out_r = out_d.rearrange("(tt p) u v -> p u tt v", tt=2)
            with (
                tc.tile_pool(name="apool", bufs=4) as apool,
                tc.tile_pool(name="rpool", bufs=4) as rpool,
                tc.tile_pool(name="opool", bufs=6) as opool,
                tc.tile_pool(name="mpsum", bufs=2, space=bass.MemorySpace.PSUM) as mpsum,
            ):
                # per-u G2 rows -> chunk-4 rhs rows 64-65 (via DRAM stage).
                # 4-deep buffering; issue each DMA 4 iterations early so it
                # never sits on the critical path (and precedes the out-DMA
                # in SP program order).
                def qdma(uu):
                    nc.sync.dma_start(
                        out=W2Q3[uu % 4][64:66, :, :],
                        in_=qstage_d[uu:uu + 1, :, :].rearrange(
                            "a (p i) v -> (a p) i v", p=2))

                for uu in range(4):
                    qdma(uu)

                for u in range(U):
                    w2q3 = W2Q3[u % 4]
                    rc4 = r8c4[u % 3]

                    # builds: r8 = |2^4 a| - 2^5 beta = max(a-b, -a-b) -> fp8
                    # (abs_max isn't a valid ISA ts-op; use the max-of-two-
                    #  linear-forms identity: P = a4-b, Q = -a4-b, max).
                    # Emission order: Pool ops first, then DVE's own P/Q,
                    # then maxes with DVE-local inputs, Pool-dependent last.
                    r8p = []
                    for p in range(2):
                        t = rpool.tile([128, 2, T], fp8, tag=f"r8p{p}")
                        r8p.append(t)
                    pPs, pQs = {}, {}
                    for c in range(2):   # Pool: P/Q for chunks 0,1
                        pP = apool.tile([128, T], bf16, tag=f"pP_{c}")
                        nc.gpsimd.tensor_scalar(pP[:], hfT4[c][:],
                                                cPm[c][:, u:u + 1], None, add)
                        pQ = apool.tile([128, T], bf16, tag=f"pQ_{c}")
                        nc.gpsimd.tensor_scalar(pQ[:], nhfT4[c][:],
                                                cQn[c][:, u:u + 1], None, add)
                        pPs[c], pQs[c] = pP, pQ
                    for c in range(2, 4):  # DVE: P/Q for chunks 2,3
                        pP = apool.tile([128, T], bf16, tag=f"pP_{c}")
                        nc.vector.tensor_scalar(pP[:], hfT4[c][:],
                                                cPm[c][:, u:u + 1], None, add)
                        pQ = apool.tile([128, T], bf16, tag=f"pQ_{c}")
                        nc.vector.tensor_scalar(pQ[:], nhfT4[c][:],
                                                cQn[c][:, u:u + 1], None, add)
                        pPs[c], pQs[c] = pP, pQ
                    for i in range(2):     # DVE: P/Q for chunk-4 halves
                        pP = apool.tile([64, T], bf16, tag=f"pP4_{i}")
                        nc.vector.tensor_scalar(pP[:], hfT4h[i][:],
                                                hgT4h[i][:, u:u + 1], None, add)
                        pQ = apool.tile([64, T], bf16, tag=f"pQ4_{i}")
                        nc.vector.tensor_scalar(pQ[:], nhfT4h[i][:],
                                                cQn4[i][:, u:u + 1], None, add)
                        pPs[4 + i], pQs[4 + i] = pP, pQ
                    for c in (2, 3):       # maxes with DVE-local inputs first
                        nc.vector.tensor_tensor(r8p[1][:, c % 2, :],
                                                pPs[c][:], pQs[c][:], mx)
                    for i in range(2):
                        nc.vector.tensor_tensor(rc4[0:64, i, :],
                                                pPs[4 + i][:], pQs[4 + i][:],
                                                mx)
                    for c in (0, 1):       # Pool-dependent maxes last
                        nc.vector.tensor_tensor(r8p[0][:, c % 2, :],
                                                pPs[c][:], pQs[c][:], mx)

                    # matmuls + drains (psum spans 2 banks; drain 1024-wide).
                    # Accumulation starts with pair (2,3) whose r8 lands first.
                    ot = opool.tile([128, 2, V], bf16, tag="ot")
                    ps = mpsum.tile([128, 2, V], f32, tag="ps")
                    for tt in range(2):
                        ts_ = slice(tt * 128, (tt + 1) * 128)
                        for vh in range(2):
                            vs = slice(vh * 512, (vh + 1) * 512)
                            nc.tensor.matmul(ps[:, tt, vs], r8p[1][:, :, ts_],
                                             W2Qp[1][:, :, vs], start=True,
                                             stop=False, perf_mode=DR)
                            nc.tensor.matmul(ps[:, tt, vs], rc4[:, :, ts_],
                                             w2q3[:, :, vs], start=False,
                                             stop=False, perf_mode=DR)
                            nc.tensor.matmul(ps[:, tt, vs], Ipair[:],
                                             F2pair[tt][:, :, vs], start=False,
                                             stop=False, perf_mode=DR)
                            nc.tensor.matmul(ps[:, tt, vs], r8p[0][:, :, ts_],
                                             W2Qp[0][:, :, vs], start=False,
                                             stop=True, perf_mode=DR)
                    nc.scalar.activation(ot[:], ps[:], Copy, bias=0.0,
                                         scale=1.0)
                    if u + 4 < U:
                        qdma(u + 4)
                    nc.sync.dma_start(out=out_r[:, u, :, :], in_=ot[:])
    nc.compile()
    return nc


def _get_nc():
    if "nc" not in _CACHE:
        _CACHE["nc"] = _build_nc()
    return _CACHE["nc"]


def run(f, g, W1, b1, W2, b2, trace=False):
    """Returns (full_output, BassKernelResults)."""
    from concourse.bass_utils import run_bass_kernel_spmd

    nc = _get_nc()
    in_maps = []
    for i in range(N_CORES):
        in_maps.append({
            "f": np.ascontiguousarray(f[i], dtype=np.float32),
            "g": np.ascontiguousarray(g[i], dtype=np.float32),
            "W1": np.ascontiguousarray(W1, dtype=np.float32),
            "b1": np.ascontiguousarray(b1, dtype=np.float32),
            "W2": np.ascontiguousarray(W2, dtype=np.float32),
            "b2": np.ascontiguousarray(b2, dtype=np.float32),
        })
    res = run_bass_kernel_spmd(nc, in_maps, list(range(N_CORES)), trace=trace)
    out = np.stack([res.results[i]["out"].astype(np.float32) * PSC
                    for i in range(N_CORES)], axis=0)
    return out, res


def kernel(f, g, W1, b1, W2, b2):
    out, _ = run(f, g, W1, b1, W2, b2)
    return out
